# revision 13
# baseline (speedup 1.0000x reference)
"""Trainium2 Bass kernel for the BrainLayer echo-state recurrence.

Reference semantics (fp32):
    proj = einsum('btf,rf->tbr', inputs, input_weights); proj[:,:,R/2:] = 0
    h_0given = reservoir_start broadcast to [B, R]
    h_t = 0.05*h_{t-1} + 0.95*tanh(h_{t-1} @ W^T + proj_t + bias)
    out  = h[:, :, R/2:]            # [B, T, R/2]
with B=16, T=1024, F=128, R=2048.

Single NeuronCore recurrence replicated on all 8 cores (the T-sequential
matrix-vector recurrence is bound by streaming W through the PE array; data
parallel sharding buys nothing and per-step collectives have a ~5-10us
floor).  Core 0's output is returned.

Key structure (v2 — PE-sequencer-bound fixes):
  * state kept transposed+scaled: s = h/0.95, W' = 0.95*W
  * pre-activation feedback form:
       z(t) = 0.05*z(t-1) + W' @ tanhT(t-1) + u'(t) + 0.95*bias
    where u'(t) = (x(t) - 0.05*x(t-1)) @ Win^T
  * u' + bias is folded into the DVE feedback op: the host precomputes
    ub(t) = 0.95*bias_A + u'_A(t+1) per step ([128,256] fp16, streamed by
    DMA) and zsbA = 0.05*zA + ub(t); no input-projection matmuls on device
  * z accumulated in PSUM by 4-way column-tiled fp16 matmuls (4 concurrent
    256-row streams, tile_position=(0,32q)); i = 128J + 32q + s lands at
    psum[32q+b, 32J+s]; halves A (i<1024) / B (i>=1024) in separate banks
    so each half's tanh -> 32x32 transpose chain overlaps the other's
    matmuls
  * ONE full-width LDWEIGHTS per 4-matmul group (stationary replicated
    into the 4 column-tiles via a 0-stride AP), emitted one group ahead;
    the per-matmul auto LDWEIGHTS are deleted post-build.  This cuts the
    PE instruction stream from ~280 to ~170 per step — the NX sequencer
    decode rate (~17ns/instr) was the binding constraint.
  * y = 0.95*(0.05*s(t-1)+tanh)[half B] staged fp32 and DMA'd per step
  * T processed in chunks of one compiled NEFF; carried state via DRAM
"""
import sys
import types
import numpy as np

B, T, F, R = 16, 1024, 128, 2048
GAMMA = 0.95
HALF = R // 2
NJ = 16
NQ = 4
NJB = 16
HN = 256
OE = 32768
OBT = 32896
CC = 33408
NSTATE = 6 * HN
CHUNK = 256
NCORES = 8

_cache = {}


def _install_ntff_shim():
    if 'antenv.axon_hooks' in sys.modules:
        return
    try:
        import antenv.axon_hooks  # noqa: F401
        return
    except Exception:
        pass
    mod = types.ModuleType('antenv.axon_hooks')
    mod._hook = None

    def set_axon_ntff_profile_hook(h):
        mod._hook = h

    def get_axon_ntff_profile_hook():
        if mod._hook is None:
            try:
                from trn_agent_boot.trn_boot import _ntff_profile_via_ctypes
                mod._hook = _ntff_profile_via_ctypes('/opt/axon/libaxon_pjrt.so')
            except Exception:
                return None
        return mod._hook

    mod.set_axon_ntff_profile_hook = set_axon_ntff_profile_hook
    mod.get_axon_ntff_profile_hook = get_axon_ntff_profile_hook
    sys.modules['antenv.axon_hooks'] = mod


def _host_prepare(x, Win, W, bias, rs):
    NP16 = np.float16
    x = np.ascontiguousarray(x, dtype=np.float32)
    Win = np.ascontiguousarray(Win, dtype=np.float32)
    W = np.ascontiguousarray(W, dtype=np.float32)
    bias = np.ascontiguousarray(bias, dtype=np.float32)
    rs = np.ascontiguousarray(rs, dtype=np.float32)

    Wp = GAMMA * W
    W4 = Wp.reshape(NJB, NQ, 32, NJ, 128)
    w_dev = np.ascontiguousarray(W4.transpose(4, 3, 1, 0, 2)).reshape(128, NJ * R)

    # E2 [128, 128]: E2[p, 32q+b] = 1 iff p == 32q+b and b < 16
    E2 = np.zeros((128, 128), dtype=np.float32)
    for q in range(NQ):
        for b in range(16):
            E2[32 * q + b, 32 * q + b] = 1.0

    arr = (0.95 * bias).reshape(NJB, NQ, 32).transpose(1, 0, 2)
    biasT95 = np.repeat(arr.reshape(NQ, 1, 512), 32, axis=1).reshape(128, 512)

    # x correction folded on host: xp(t) = x(t) - 0.05*x(t-1)
    xp = x.copy()
    xp[:, 1:, :] -= 0.05 * x[:, :-1, :]

    # u'(t) = xp(t) @ Win_A^T, half A only, in psum layout
    # psum[32q+b, 32J+s] = u'[b, i=128J+32q+s]
    WinA = Win[:HALF]                              # [1024, F]
    u = xp.reshape(B * T, F) @ WinA.T              # [B*T, 1024]
    u4 = u.reshape(B, T, 8, NQ, 32)                # [b, t, J, q, s]
    U = np.zeros((T + 1, NQ, 32, 8, 32), dtype=np.float32)
    U[:T, :, :16] = u4.transpose(1, 3, 0, 2, 4)    # [t, q, b, J, s]
    U = U.reshape(T + 1, 128, 256)

    # per-step DVE feedback operand: ub(t) = 0.95*bias_A + u'_A(t+1)
    ub_full = (biasT95[None, :, 0:HN] + U[1:T + 1]).astype(NP16)

    s0 = (rs / GAMMA).reshape(NJB, NQ, 32)
    s0T = np.ascontiguousarray(
        np.broadcast_to(s0.transpose(1, 2, 0)[:, :, :, None], (NQ, 32, NJB, 32))
    ).reshape(128, 512)

    const = np.zeros((128, CC), dtype=NP16)
    const[:, 0:32768] = w_dev.astype(NP16)
    const[:, OE:OE + 128] = E2.astype(NP16)
    const[:, OBT:OBT + 512] = biasT95.astype(NP16)

    # initial carried state
    arrb = bias.reshape(NJB, NQ, 32).transpose(1, 0, 2)
    biasT = np.repeat(arrb.reshape(NQ, 1, 512), 32, axis=1).reshape(128, 512)
    st = np.zeros((128, NSTATE), dtype=NP16)
    st[:, 0:HN] = s0T[:, 0:HN].astype(NP16)
    st[:, HN:2 * HN] = s0T[:, HN:2 * HN].astype(NP16)
    st[:, 2 * HN:3 * HN] = (biasT[:, 0:HN] + U[0]).astype(NP16)
    st[:, 3 * HN:4 * HN] = biasT[:, HN:2 * HN].astype(NP16)
    st[:, 4 * HN:5 * HN] = s0T[:, HN:2 * HN].astype(NP16)
    return {"const": const, "ub": ub_full}, st


def _thin_matmul_updates(nc):
    """Strip the PE completion-semaphore increment from matmuls 0-2 of each
    4-matmul group, keeping the increment on the 4th (matmuls complete in
    pc order, so the group-end increment subsumes the others).  All waits
    on that semaphore are rewritten from matmul counts to group counts
    (ceil: rounding up is always dependency-safe).

    Rationale (from HW traces): each sem increment costs ~26ns of PE
    sequencer/commit serialization.  At 140 matmuls/step that alone is
    ~3.7us/step and starves the engine queue into a degraded issue mode
    (136ns/group instead of the 109ns 4-stream rate).
    """
    import bass_rust
    from collections import Counter

    # identify the PE completion semaphore: the one incremented by matmuls
    cnt = Counter()
    for f in nc.m.functions:
        for bb in f.blocks:
            for inst in bb.instructions:
                if str(inst.opcode) == 'Matmult' and inst.sync_info:
                    for u in inst.sync_info.on_update:
                        if u.sync_type == 'semaphore' and \
                                u.update_mode == 'sem-inc':
                            cnt[u.id] += 1
    if not cnt:
        return 0
    pe_sem, n_mm_inc = cnt.most_common(1)[0]

    # collect all wait thresholds on the PE sem (matmul-count values)
    targets = set()
    for f in nc.m.functions:
        for bb in f.blocks:
            for inst in bb.instructions:
                si = inst.sync_info
                if si is None:
                    continue
                for w in si.on_wait:
                    if w.sync_type == 'semaphore' and w.id == pe_sem:
                        assert w.wait_mode == 'sem-ge-imm', w.wait_mode
                        targets.add(w.wait_value)

    # keep the increment on: every matmul that is a wait target, and the
    # last matmul of each 4-group; strip the rest.  Build the exact
    # old-count -> new-count map for threshold rewriting.
    n_strip = 0
    value_map = {}
    k = 0          # 1-indexed matmul count after this matmul completes
    kept = 0
    for f in nc.m.functions:
        for bb in f.blocks:
            for inst in bb.instructions:
                if str(inst.opcode) != 'Matmult':
                    continue
                k += 1
                keep = (k in targets) or (k % 4 == 0)
                if keep:
                    kept += 1
                    value_map[k] = kept
                else:
                    si = inst.sync_info
                    if si is not None:
                        ups = [u for u in si.on_update
                               if not (u.sync_type == 'semaphore'
                                       and u.id == pe_sem)]
                        if len(ups) != len(si.on_update):
                            n_strip += 1
                            inst.sync_info = bass_rust.SyncInfo(
                                on_wait=list(si.on_wait), on_update=ups)

    for f in nc.m.functions:
        for bb in f.blocks:
            for inst in bb.instructions:
                si = inst.sync_info
                if si is None:
                    continue
                changed = False
                waits = []
                for w in si.on_wait:
                    if w.sync_type == 'semaphore' and w.id == pe_sem:
                        nv = value_map.get(w.wait_value, w.wait_value)
                        assert w.wait_value in value_map or w.wait_value <= 0
                        waits.append(bass_rust.SyncWait(
                            sync_type='semaphore', id=w.id,
                            ant_name=w.ant_name, wait_mode=w.wait_mode,
                            wait_value=nv, wait_reg=None))
                        changed = True
                    else:
                        waits.append(w)
                if changed:
                    inst.sync_info = bass_rust.SyncInfo(
                        on_wait=waits, on_update=list(si.on_update))
    return n_strip


def _legalize_waits(nc, mybir, keep=1):
    """Walrus here encodes only ~1 sync wait per instruction; split extras
    onto same-engine NoOps."""
    import bass_rust
    ctr = 0
    for f in nc.m.functions:
        for bb in f.blocks:
            out = []
            for inst in bb.instructions:
                si = inst.sync_info
                if si is not None and len(si.on_wait) > keep:
                    waits = list(si.on_wait)
                    extra, kept = waits[:-keep], waits[-keep:]
                    for w in extra:
                        ctr += 1
                        out.append(mybir.InstNoOp(
                            name=f"I-wgate-{ctr}", engine=inst.engine,
                            sync_info=bass_rust.SyncInfo(on_wait=[w],
                                                         on_update=[]),
                        ))
                    inst.sync_info = bass_rust.SyncInfo(
                        on_wait=kept, on_update=list(si.on_update))
                out.append(inst)
            bb.instructions = out
    return ctr


def _build(nsteps):
    import concourse.bass as bass
    import concourse.mybir as mybir
    from concourse.tile import TileContext

    FP32 = mybir.dt.float32
    FP16 = mybir.dt.float16
    nc = bass.Bass()

    ub_d = nc.declare_dram_parameter("ub", [nsteps, 128, HN], FP16,
                                     isOutput=False)
    const_d = nc.declare_dram_parameter("const", [128, CC], FP16,
                                        isOutput=False)
    st_d = nc.declare_dram_parameter("state_in", [128, NSTATE], FP16,
                                     isOutput=False)
    y_d = nc.declare_dram_parameter("y", [nsteps, 128, 128], FP32,
                                    isOutput=True)
    sto_d = nc.declare_dram_parameter("state_out", [128, NSTATE], FP16,
                                      isOutput=True)

    with TileContext(nc) as tc:
        with (
            tc.tile_pool(name="const", bufs=1) as cpool,
            tc.tile_pool(name="state", bufs=2) as spool,
            tc.tile_pool(name="ttp", bufs=2) as tpool,
            tc.tile_pool(name="zsb", bufs=2) as zspool,
            tc.tile_pool(name="work", bufs=2) as wpool,
            tc.tile_pool(name="ubin", bufs=4) as ubpool,
            tc.tile_pool(name="yout", bufs=4) as ypool,
            tc.tile_pool(name="psum", bufs=2, space="PSUM") as ppool,
        ):
            const_sb = cpool.tile([128, CC], FP16, tag="const")
            col = 0
            for w_cols in [4096] * 8 + [CC - 8 * 4096]:
                nc.sync.dma_start(out=const_sb[:, col:col + w_cols],
                                  in_=const_d[:, col:col + w_cols])
                col += w_cols

            tTA = tpool.tile([128, HN], FP16, tag="tTA")
            nc.sync.dma_start(out=tTA[:, :], in_=st_d[:, 0:HN])
            tTB = tpool.tile([128, HN], FP16, tag="tTB")
            nc.sync.dma_start(out=tTB[:, :], in_=st_d[:, HN:2 * HN])
            zSBA = zspool.tile([128, HN], FP16, tag="zSBA")
            nc.sync.dma_start(out=zSBA[:, :], in_=st_d[:, 2 * HN:3 * HN])
            zSBB = zspool.tile([128, HN], FP16, tag="zSBB")
            nc.sync.dma_start(out=zSBB[:, :], in_=st_d[:, 3 * HN:4 * HN])
            sB = spool.tile([128, HN], FP16, tag="sB")
            nc.sync.dma_start(out=sB[:, :], in_=st_d[:, 4 * HN:5 * HN])

            prev = {"tTA": tTA, "tTB": tTB, "zSBA": zSBA, "zSBB": zSBB}

            # zero psum slots once: rows b>=16 of each strip are never written
            # by matmuls but are read by the zSB feedback copy
            for tag in ("zA", "zA", "zB", "zB"):
                ztmp = ppool.tile([128, HN], FP32, tag=tag)
                nc.vector.memset(ztmp[:, :], 0.0)

            for step in range(nsteps):
                ub_t = ubpool.tile([128, HN], FP16, tag="ub")
                nc.sync.dma_start(out=ub_t[:, :], in_=ub_d[step])

                zA = ppool.tile([128, HN], FP32, tag="zA")
                zB = ppool.tile([128, HN], FP32, tag="zB")

                def wgroup(z, ho, jt, start=False, stop=False):
                    src = prev["tTA"] if jt < 8 else prev["tTB"]
                    c = 32 * (jt % 8)
                    for q in range(NQ):
                        nc.tensor.matmul(
                            z[32 * q:32 * q + 16, :],
                            src[:, c:c + 16],
                            const_sb[:, R * jt + 512 * q + ho:
                                     R * jt + 512 * q + ho + HN],
                            start=start, stop=stop,
                            tile_position=(0, 32 * q),
                        )

                def zinj(z, zsb_prev):
                    for q in range(NQ):
                        nc.tensor.matmul(
                            z[32 * q:32 * q + 16, :],
                            const_sb[:, OE + 32 * q:OE + 32 * q + 16],
                            zsb_prev[:, :],
                            start=False, stop=False,
                            tile_position=(0, 32 * q),
                        )

                def tail(half, z, bias_ap):
                    tt = wpool.tile([128, HN], FP16, tag="tt" + half)
                    tT = tpool.tile([128, HN], FP16, tag="tT" + half)
                    # segment so ACT/DVE pipeline: first 4 stationary
                    # tiles unblock the next step's consumers earlier
                    HH = HN // 2
                    nc.scalar.activation(tt[:, 0:HH], z[:, 0:HH],
                                         mybir.ActivationFunctionType.Tanh)
                    nc.vector.transpose(tT[:, 0:HH], tt[:, 0:HH])
                    nc.scalar.activation(tt[:, HH:HN], z[:, HH:HN],
                                         mybir.ActivationFunctionType.Tanh)
                    nc.vector.transpose(tT[:, HH:HN], tt[:, HH:HN])
                    zsb = zspool.tile([128, HN], FP16, tag="zSB" + half)
                    nc.vector.scalar_tensor_tensor(
                        zsb[:, :], z[:, :], 1.0 - GAMMA, bias_ap,
                        mybir.AluOpType.mult, mybir.AluOpType.add,
                    )
                    return tT, zsb

                # Wave order: [A-j0..7 zinjA A-j8..15] [B-j0..7 zinjB
                # B-j8..15] so each half's stop-wave lands ~half a step
                # before the consumers of its tanh-transpose output.
                # Wave order: the jt>=8 waves (consuming the freshest half of
                # the previous state, tTB(t-1), produced at the very end of
                # step t-1) are pushed ~18 groups into the step so the
                # tanh->transpose chain has ~2us of slack.
                wgroup(zA, 0, 0, start=True)
                for jt in range(1, 8):
                    wgroup(zA, 0, jt)
                zinj(zA, prev["zSBA"])
                wgroup(zB, HN, 0, start=True)
                for jt in range(1, 8):
                    wgroup(zB, HN, jt)
                for jt in range(8, NJ):
                    wgroup(zA, 0, jt, stop=(jt == NJ - 1))
                tTA, zSBA = tail("A", zA, ub_t[:, :])
                zinj(zB, prev["zSBB"])
                for jt in range(8, NJ):
                    wgroup(zB, HN, jt, stop=(jt == NJ - 1))
                tTB, zSBB = tail("B", zB, const_sb[:, OBT + HN:OBT + 2 * HN])

                sB_new = spool.tile([128, HN], FP16, tag="sB")
                nc.vector.scalar_tensor_tensor(
                    sB_new[:, :], sB[:, :], 1.0 - GAMMA, tTB[:, :],
                    mybir.AluOpType.mult, mybir.AluOpType.add,
                )
                y_stage = ypool.tile([128, 128], FP32, tag="y")
                nc.vector.tensor_scalar_mul(
                    y_stage[:, :].rearrange("p (J b) -> p J b", b=16),
                    sB_new[:, :].rearrange("p (J b) -> p J b", b=32)[:, :, 0:16],
                    GAMMA,
                )
                nc.sync.dma_start(
                    out=bass.AP(y_d, step * 128 * 128, [[128, 128], [1, 128]]),
                    in_=y_stage[:, :],
                )
                sB = sB_new
                prev = {"tTA": tTA, "tTB": tTB, "zSBA": zSBA, "zSBB": zSBB}

            nc.sync.dma_start(out=sto_d[:, 0:HN], in_=prev["tTA"][:, :])
            nc.sync.dma_start(out=sto_d[:, HN:2 * HN], in_=prev["tTB"][:, :])
            nc.sync.dma_start(out=sto_d[:, 2 * HN:3 * HN], in_=prev["zSBA"][:, :])
            nc.sync.dma_start(out=sto_d[:, 3 * HN:4 * HN], in_=prev["zSBB"][:, :])
            nc.sync.dma_start(out=sto_d[:, 4 * HN:5 * HN], in_=sB[:, :])

    _thin_matmul_updates(nc)
    _legalize_waits(nc, mybir)
    return nc


def run_kernel(inputs, input_weights, recurrent_weights, bias,
               reservoir_start, trace=False):
    """Run the full T; returns (y [B,T,HALF] fp32, hw_ns or None)."""
    _install_ntff_shim()
    from concourse.bass_utils import run_bass_kernel_spmd

    dev_inputs, state = _host_prepare(inputs, input_weights,
                                      recurrent_weights, bias,
                                      reservoir_start)
    if "nc" not in _cache:
        _cache["nc"] = _build(CHUNK)
    nc = _cache["nc"]

    core_ids = list(range(NCORES))
    ys = []
    total_ns = 0
    have_ns = True
    for c0 in range(0, T, CHUNK):
        in_map = {"ub": np.ascontiguousarray(dev_inputs["ub"][c0:c0 + CHUNK]),
                  "const": dev_inputs["const"], "state_in": state}
        res = run_bass_kernel_spmd(nc, [dict(in_map) for _ in core_ids],
                                   core_ids, trace=trace)
        ys.append(res.results[0]["y"])
        state = np.ascontiguousarray(res.results[0]["state_out"])
        if res.exec_time_ns is not None:
            total_ns += res.exec_time_ns
        else:
            have_ns = False
    y_dev = np.concatenate(ys, axis=0)  # [T, 128, 128]
    y = np.ascontiguousarray(
        y_dev.reshape(T, 128, 8, 16).transpose(3, 0, 2, 1)
    ).reshape(B, T, HALF).astype(np.float32)
    return y, (total_ns if have_ns else None)


def kernel(inputs, input_weights, recurrent_weights, bias, reservoir_start):
    y, _ = run_kernel(inputs, input_weights, recurrent_weights, bias,
                      reservoir_start, trace=False)
    return y


# revision 16
# speedup vs baseline: 1.0258x; 1.0258x over previous
"""Trainium2 Bass kernel for the BrainLayer echo-state recurrence.

Reference semantics (fp32):
    proj = einsum('btf,rf->tbr', inputs, input_weights); proj[:,:,R/2:] = 0
    h_0given = reservoir_start broadcast to [B, R]
    h_t = 0.05*h_{t-1} + 0.95*tanh(h_{t-1} @ W^T + proj_t + bias)
    out  = h[:, :, R/2:]            # [B, T, R/2]
with B=16, T=1024, F=128, R=2048.

Single NeuronCore recurrence replicated on all 8 cores (the T-sequential
matrix-vector recurrence is bound by streaming W through the PE array; data
parallel sharding buys nothing and per-step collectives have a ~5-10us
floor).  Core 0's output is returned.

Key structure (v2 — PE-sequencer-bound fixes):
  * state kept transposed+scaled: s = h/0.95, W' = 0.95*W
  * pre-activation feedback form:
       z(t) = 0.05*z(t-1) + W' @ tanhT(t-1) + u'(t) + 0.95*bias
    where u'(t) = (x(t) - 0.05*x(t-1)) @ Win^T
  * u' + bias is folded into the DVE feedback op: the host precomputes
    ub(t) = 0.95*bias_A + u'_A(t+1) per step ([128,256] fp16, streamed by
    DMA) and zsbA = 0.05*zA + ub(t); no input-projection matmuls on device
  * z accumulated in PSUM by 4-way column-tiled fp16 matmuls (4 concurrent
    256-row streams, tile_position=(0,32q)); i = 128J + 32q + s lands at
    psum[32q+b, 32J+s]; halves A (i<1024) / B (i>=1024) in separate banks
    so each half's tanh -> 32x32 transpose chain overlaps the other's
    matmuls
  * PE completion-semaphore increments stripped from 3 of every 4 matmuls
    post-build (_thin_matmul_updates), with wait thresholds remapped
    exactly.  The per-matmul sem-inc commit (~26ns each) starved the PE
    queue into a degraded issue mode (136ns/group vs the 109ns 4-stream
    rate); with 1 inc per group the groups sustain the stream rate.
  * y = 0.95*(0.05*s(t-1)+tanh)[half B] staged fp32 and DMA'd per step
  * T processed in chunks of one compiled NEFF; carried state via DRAM
"""
import sys
import types
import numpy as np

B, T, F, R = 16, 1024, 128, 2048
GAMMA = 0.95
HALF = R // 2
NJ = 16
NQ = 4
NJB = 16
HN = 256
OE = 32768
OBT = 32896
CC = 33408
NSTATE = 6 * HN
CHUNK = 256
NCORES = 8

_cache = {}


def _install_ntff_shim():
    if 'antenv.axon_hooks' in sys.modules:
        return
    try:
        import antenv.axon_hooks  # noqa: F401
        return
    except Exception:
        pass
    mod = types.ModuleType('antenv.axon_hooks')
    mod._hook = None

    def set_axon_ntff_profile_hook(h):
        mod._hook = h

    def get_axon_ntff_profile_hook():
        if mod._hook is None:
            try:
                from trn_agent_boot.trn_boot import _ntff_profile_via_ctypes
                mod._hook = _ntff_profile_via_ctypes('/opt/axon/libaxon_pjrt.so')
            except Exception:
                return None
        return mod._hook

    mod.set_axon_ntff_profile_hook = set_axon_ntff_profile_hook
    mod.get_axon_ntff_profile_hook = get_axon_ntff_profile_hook
    sys.modules['antenv.axon_hooks'] = mod


def _host_prepare(x, Win, W, bias, rs):
    NP16 = np.float16
    x = np.ascontiguousarray(x, dtype=np.float32)
    Win = np.ascontiguousarray(Win, dtype=np.float32)
    W = np.ascontiguousarray(W, dtype=np.float32)
    bias = np.ascontiguousarray(bias, dtype=np.float32)
    rs = np.ascontiguousarray(rs, dtype=np.float32)

    Wp = GAMMA * W
    W4 = Wp.reshape(NJB, NQ, 32, NJ, 128)
    w_dev = np.ascontiguousarray(W4.transpose(4, 3, 1, 0, 2)).reshape(128, NJ * R)

    # E2 [128, 128]: E2[p, 32q+b] = 1 iff p == 32q+b and b < 16
    E2 = np.zeros((128, 128), dtype=np.float32)
    for q in range(NQ):
        for b in range(16):
            E2[32 * q + b, 32 * q + b] = 1.0

    arr = (0.95 * bias).reshape(NJB, NQ, 32).transpose(1, 0, 2)
    biasT95 = np.repeat(arr.reshape(NQ, 1, 512), 32, axis=1).reshape(128, 512)

    # x correction folded on host: xp(t) = x(t) - 0.05*x(t-1)
    xp = x.copy()
    xp[:, 1:, :] -= 0.05 * x[:, :-1, :]

    # u'(t) = xp(t) @ Win_A^T, half A only, in psum layout
    # psum[32q+b, 32J+s] = u'[b, i=128J+32q+s]
    WinA = Win[:HALF]                              # [1024, F]
    u = xp.reshape(B * T, F) @ WinA.T              # [B*T, 1024]
    u4 = u.reshape(B, T, 8, NQ, 32)                # [b, t, J, q, s]
    U = np.zeros((T + 1, NQ, 32, 8, 32), dtype=np.float32)
    U[:T, :, :16] = u4.transpose(1, 3, 0, 2, 4)    # [t, q, b, J, s]
    U = U.reshape(T + 1, 128, 256)

    # per-step DVE feedback operand: ub(t) = 0.95*bias_A + u'_A(t+1)
    ub_full = (biasT95[None, :, 0:HN] + U[1:T + 1]).astype(NP16)

    s0 = (rs / GAMMA).reshape(NJB, NQ, 32)
    s0T = np.ascontiguousarray(
        np.broadcast_to(s0.transpose(1, 2, 0)[:, :, :, None], (NQ, 32, NJB, 32))
    ).reshape(128, 512)

    const = np.zeros((128, CC), dtype=NP16)
    const[:, 0:32768] = w_dev.astype(NP16)
    const[:, OE:OE + 128] = E2.astype(NP16)
    const[:, OBT:OBT + 512] = biasT95.astype(NP16)

    # initial carried state
    arrb = bias.reshape(NJB, NQ, 32).transpose(1, 0, 2)
    biasT = np.repeat(arrb.reshape(NQ, 1, 512), 32, axis=1).reshape(128, 512)
    st = np.zeros((128, NSTATE), dtype=NP16)
    st[:, 0:HN] = s0T[:, 0:HN].astype(NP16)
    st[:, HN:2 * HN] = s0T[:, HN:2 * HN].astype(NP16)
    st[:, 2 * HN:3 * HN] = (biasT[:, 0:HN] + U[0]).astype(NP16)
    st[:, 3 * HN:4 * HN] = biasT[:, HN:2 * HN].astype(NP16)
    st[:, 4 * HN:5 * HN] = s0T[:, HN:2 * HN].astype(NP16)
    return {"const": const, "ub": ub_full}, st


def _thin_matmul_updates(nc):
    """Strip the PE completion-semaphore increment from matmuls 0-2 of each
    4-matmul group, keeping the increment on the 4th (matmuls complete in
    pc order, so the group-end increment subsumes the others).  All waits
    on that semaphore are rewritten from matmul counts to group counts
    (ceil: rounding up is always dependency-safe).

    Rationale (from HW traces): each sem increment costs ~26ns of PE
    sequencer/commit serialization.  At 140 matmuls/step that alone is
    ~3.7us/step and starves the engine queue into a degraded issue mode
    (136ns/group instead of the 109ns 4-stream rate).
    """
    import bass_rust
    from collections import Counter

    # identify the PE completion semaphore: the one incremented by matmuls
    cnt = Counter()
    for f in nc.m.functions:
        for bb in f.blocks:
            for inst in bb.instructions:
                if str(inst.opcode) == 'Matmult' and inst.sync_info:
                    for u in inst.sync_info.on_update:
                        if u.sync_type == 'semaphore' and \
                                u.update_mode == 'sem-inc':
                            cnt[u.id] += 1
    if not cnt:
        return 0
    pe_sem, n_mm_inc = cnt.most_common(1)[0]

    # collect all wait thresholds on the PE sem (matmul-count values)
    targets = set()
    for f in nc.m.functions:
        for bb in f.blocks:
            for inst in bb.instructions:
                si = inst.sync_info
                if si is None:
                    continue
                for w in si.on_wait:
                    if w.sync_type == 'semaphore' and w.id == pe_sem:
                        assert w.wait_mode == 'sem-ge-imm', w.wait_mode
                        targets.add(w.wait_value)

    # keep the increment on: every matmul that is a wait target, and the
    # last matmul of each 4-group; strip the rest.  Build the exact
    # old-count -> new-count map for threshold rewriting.
    n_strip = 0
    value_map = {}
    k = 0          # 1-indexed matmul count after this matmul completes
    kept = 0
    for f in nc.m.functions:
        for bb in f.blocks:
            for inst in bb.instructions:
                if str(inst.opcode) != 'Matmult':
                    continue
                k += 1
                keep = (k in targets) or (k % 4 == 0)
                if keep:
                    kept += 1
                    value_map[k] = kept
                else:
                    si = inst.sync_info
                    if si is not None:
                        ups = [u for u in si.on_update
                               if not (u.sync_type == 'semaphore'
                                       and u.id == pe_sem)]
                        if len(ups) != len(si.on_update):
                            n_strip += 1
                            inst.sync_info = bass_rust.SyncInfo(
                                on_wait=list(si.on_wait), on_update=ups)

    for f in nc.m.functions:
        for bb in f.blocks:
            for inst in bb.instructions:
                si = inst.sync_info
                if si is None:
                    continue
                changed = False
                waits = []
                for w in si.on_wait:
                    if w.sync_type == 'semaphore' and w.id == pe_sem:
                        nv = value_map.get(w.wait_value, w.wait_value)
                        assert w.wait_value in value_map or w.wait_value <= 0
                        waits.append(bass_rust.SyncWait(
                            sync_type='semaphore', id=w.id,
                            ant_name=w.ant_name, wait_mode=w.wait_mode,
                            wait_value=nv, wait_reg=None))
                        changed = True
                    else:
                        waits.append(w)
                if changed:
                    inst.sync_info = bass_rust.SyncInfo(
                        on_wait=waits, on_update=list(si.on_update))
    return n_strip


def _legalize_waits(nc, mybir, keep=1):
    """Walrus here encodes only ~1 sync wait per instruction; split extras
    onto same-engine NoOps."""
    import bass_rust
    ctr = 0
    for f in nc.m.functions:
        for bb in f.blocks:
            out = []
            for inst in bb.instructions:
                si = inst.sync_info
                if si is not None and len(si.on_wait) > keep:
                    waits = list(si.on_wait)
                    extra, kept = waits[:-keep], waits[-keep:]
                    for w in extra:
                        ctr += 1
                        out.append(mybir.InstNoOp(
                            name=f"I-wgate-{ctr}", engine=inst.engine,
                            sync_info=bass_rust.SyncInfo(on_wait=[w],
                                                         on_update=[]),
                        ))
                    inst.sync_info = bass_rust.SyncInfo(
                        on_wait=kept, on_update=list(si.on_update))
                out.append(inst)
            bb.instructions = out
    return ctr


def _build(nsteps):
    import concourse.bass as bass
    import concourse.mybir as mybir
    from concourse.tile import TileContext

    FP32 = mybir.dt.float32
    FP16 = mybir.dt.float16
    nc = bass.Bass()

    ub_d = nc.declare_dram_parameter("ub", [nsteps, 128, HN], FP16,
                                     isOutput=False)
    const_d = nc.declare_dram_parameter("const", [128, CC], FP16,
                                        isOutput=False)
    st_d = nc.declare_dram_parameter("state_in", [128, NSTATE], FP16,
                                     isOutput=False)
    y_d = nc.declare_dram_parameter("y", [nsteps, 128, 128], FP32,
                                    isOutput=True)
    sto_d = nc.declare_dram_parameter("state_out", [128, NSTATE], FP16,
                                      isOutput=True)

    with TileContext(nc) as tc:
        with (
            tc.tile_pool(name="const", bufs=1) as cpool,
            tc.tile_pool(name="state", bufs=2) as spool,
            tc.tile_pool(name="ttp", bufs=2) as tpool,
            tc.tile_pool(name="zsb", bufs=2) as zspool,
            tc.tile_pool(name="work", bufs=2) as wpool,
            tc.tile_pool(name="ubin", bufs=4) as ubpool,
            tc.tile_pool(name="yout", bufs=4) as ypool,
            tc.tile_pool(name="psum", bufs=2, space="PSUM") as ppool,
        ):
            const_sb = cpool.tile([128, CC], FP16, tag="const")
            col = 0
            for w_cols in [4096] * 8 + [CC - 8 * 4096]:
                nc.sync.dma_start(out=const_sb[:, col:col + w_cols],
                                  in_=const_d[:, col:col + w_cols])
                col += w_cols

            tTA = tpool.tile([128, HN], FP16, tag="tTA")
            nc.sync.dma_start(out=tTA[:, :], in_=st_d[:, 0:HN])
            tTB = tpool.tile([128, HN], FP16, tag="tTB")
            nc.sync.dma_start(out=tTB[:, :], in_=st_d[:, HN:2 * HN])
            zSBA = zspool.tile([128, HN], FP16, tag="zSBA")
            nc.sync.dma_start(out=zSBA[:, :], in_=st_d[:, 2 * HN:3 * HN])
            zSBB = zspool.tile([128, HN], FP16, tag="zSBB")
            nc.sync.dma_start(out=zSBB[:, :], in_=st_d[:, 3 * HN:4 * HN])
            sB = spool.tile([128, HN], FP16, tag="sB")
            nc.sync.dma_start(out=sB[:, :], in_=st_d[:, 4 * HN:5 * HN])

            prev = {"tTA": tTA, "tTB": tTB, "zSBA": zSBA, "zSBB": zSBB}

            # zero psum slots once: rows b>=16 of each strip are never written
            # by matmuls but are read by the zSB feedback copy
            for tag in ("zA", "zA", "zB", "zB"):
                ztmp = ppool.tile([128, HN], FP32, tag=tag)
                nc.vector.memset(ztmp[:, :], 0.0)

            for step in range(nsteps):
                ub_t = ubpool.tile([128, HN], FP16, tag="ub")
                nc.sync.dma_start(out=ub_t[:, :], in_=ub_d[step])

                zA = ppool.tile([128, HN], FP32, tag="zA")
                zB = ppool.tile([128, HN], FP32, tag="zB")

                def wgroup(z, ho, jt, start=False, stop=False):
                    src = prev["tTA"] if jt < 8 else prev["tTB"]
                    c = 32 * (jt % 8)
                    for q in range(NQ):
                        nc.tensor.matmul(
                            z[32 * q:32 * q + 16, :],
                            src[:, c:c + 16],
                            const_sb[:, R * jt + 512 * q + ho:
                                     R * jt + 512 * q + ho + HN],
                            start=start, stop=stop,
                            tile_position=(0, 32 * q),
                        )

                def zinj(z, zsb_prev):
                    for q in range(NQ):
                        nc.tensor.matmul(
                            z[32 * q:32 * q + 16, :],
                            const_sb[:, OE + 32 * q:OE + 32 * q + 16],
                            zsb_prev[:, :],
                            start=False, stop=False,
                            tile_position=(0, 32 * q),
                        )

                def tail(half, z, bias_ap):
                    tt = wpool.tile([128, HN], FP16, tag="tt" + half)
                    tT = tpool.tile([128, HN], FP16, tag="tT" + half)
                    # segment so ACT/DVE pipeline: early pieces unblock the
                    # next step's consumer LDWEIGHTS as soon as possible.
                    # Half B feeds the jt>=8 waves whose slack is tight
                    # (produced at step end, consumed ~10 groups in), so it
                    # gets the finest pieces.
                    npc = 4 if half == "B" else 2
                    pw = HN // npc
                    for p in range(npc):
                        a, b = p * pw, (p + 1) * pw
                        nc.scalar.activation(tt[:, a:b], z[:, a:b],
                                             mybir.ActivationFunctionType.Tanh)
                        nc.vector.transpose(tT[:, a:b], tt[:, a:b])
                    zsb = zspool.tile([128, HN], FP16, tag="zSB" + half)
                    nc.vector.scalar_tensor_tensor(
                        zsb[:, :], z[:, :], 1.0 - GAMMA, bias_ap,
                        mybir.AluOpType.mult, mybir.AluOpType.add,
                    )
                    return tT, zsb

                # Wave order: [A-j0..7 zinjA A-j8..15] [B-j0..7 zinjB
                # B-j8..15] so each half's stop-wave lands ~half a step
                # before the consumers of its tanh-transpose output.
                wgroup(zA, 0, 0, start=True)
                for jt in range(1, 8):
                    wgroup(zA, 0, jt)
                zinj(zA, prev["zSBA"])
                for jt in range(8, NJ):
                    wgroup(zA, 0, jt, stop=(jt == NJ - 1))
                tTA, zSBA = tail("A", zA, ub_t[:, :])

                wgroup(zB, HN, 0, start=True)
                for jt in range(1, 8):
                    wgroup(zB, HN, jt)
                zinj(zB, prev["zSBB"])
                for jt in range(8, NJ):
                    wgroup(zB, HN, jt, stop=(jt == NJ - 1))
                tTB, zSBB = tail("B", zB, const_sb[:, OBT + HN:OBT + 2 * HN])

                sB_new = spool.tile([128, HN], FP16, tag="sB")
                nc.vector.scalar_tensor_tensor(
                    sB_new[:, :], sB[:, :], 1.0 - GAMMA, tTB[:, :],
                    mybir.AluOpType.mult, mybir.AluOpType.add,
                )
                y_stage = ypool.tile([128, 128], FP32, tag="y")
                nc.vector.tensor_scalar_mul(
                    y_stage[:, :].rearrange("p (J b) -> p J b", b=16),
                    sB_new[:, :].rearrange("p (J b) -> p J b", b=32)[:, :, 0:16],
                    GAMMA,
                )
                nc.sync.dma_start(
                    out=bass.AP(y_d, step * 128 * 128, [[128, 128], [1, 128]]),
                    in_=y_stage[:, :],
                )
                sB = sB_new
                prev = {"tTA": tTA, "tTB": tTB, "zSBA": zSBA, "zSBB": zSBB}

            nc.sync.dma_start(out=sto_d[:, 0:HN], in_=prev["tTA"][:, :])
            nc.sync.dma_start(out=sto_d[:, HN:2 * HN], in_=prev["tTB"][:, :])
            nc.sync.dma_start(out=sto_d[:, 2 * HN:3 * HN], in_=prev["zSBA"][:, :])
            nc.sync.dma_start(out=sto_d[:, 3 * HN:4 * HN], in_=prev["zSBB"][:, :])
            nc.sync.dma_start(out=sto_d[:, 4 * HN:5 * HN], in_=sB[:, :])

    _thin_matmul_updates(nc)
    _legalize_waits(nc, mybir)
    return nc


def run_kernel(inputs, input_weights, recurrent_weights, bias,
               reservoir_start, trace=False):
    """Run the full T; returns (y [B,T,HALF] fp32, hw_ns or None)."""
    _install_ntff_shim()
    from concourse.bass_utils import run_bass_kernel_spmd

    dev_inputs, state = _host_prepare(inputs, input_weights,
                                      recurrent_weights, bias,
                                      reservoir_start)
    if "nc" not in _cache:
        _cache["nc"] = _build(CHUNK)
    nc = _cache["nc"]

    core_ids = list(range(NCORES))
    ys = []
    total_ns = 0
    have_ns = True
    for c0 in range(0, T, CHUNK):
        in_map = {"ub": np.ascontiguousarray(dev_inputs["ub"][c0:c0 + CHUNK]),
                  "const": dev_inputs["const"], "state_in": state}
        res = run_bass_kernel_spmd(nc, [dict(in_map) for _ in core_ids],
                                   core_ids, trace=trace)
        ys.append(res.results[0]["y"])
        state = np.ascontiguousarray(res.results[0]["state_out"])
        if res.exec_time_ns is not None:
            total_ns += res.exec_time_ns
        else:
            have_ns = False
    y_dev = np.concatenate(ys, axis=0)  # [T, 128, 128]
    y = np.ascontiguousarray(
        y_dev.reshape(T, 128, 8, 16).transpose(3, 0, 2, 1)
    ).reshape(B, T, HALF).astype(np.float32)
    return y, (total_ns if have_ns else None)


def kernel(inputs, input_weights, recurrent_weights, bias, reservoir_start):
    y, _ = run_kernel(inputs, input_weights, recurrent_weights, bias,
                      reservoir_start, trace=False)
    return y
